# revision 1
# baseline (speedup 1.0000x reference)
"""Trainium2 Bass kernel: FlowMatching action-distribution log-prob head.

Math (per Euler step s, t_s = 1 - s*dt, dt = 1/n_steps):
    z1 = a@W1a + c@W1c + t_s*w1t + b1          (W1 split: rows 0:8 / 8:264 / 264)
    h1 = silu(z1);  dz1 = e@W1a;  dh1 = silu'(z1) * dz1
    z2 = h1@W2 + b2; h2 = silu(z2); dz2 = dh1@W2; dh2 = silu'(z2)*dz2
    v  = h2@W3 + b3; jv = dh2@W3
    a -= v*dt;  div_int += dt * sum(jv*e)
Output: logp = -0.5*||a0||^2 - 0.5*A*ln(2pi) - div_int     [B,1]

silu and derivative_silu live in different ACT table sets on TRN2 (2.7us
switch per use), so silu' is computed exactly from the silu set itself:
    T = tanh(z/2);  q = (1-T)/2;  silu'(z) = 1 + q*(silu(z)-1)
The tangent rides through W3V = -dt*W3:
    jv~ = dh2 @ W3V = -dt*jv;  tmp = jv~*e;  div_ps += (-1)*sum_p tmp = +dt*div_s

Layout: feature-major. Activations [features(part), batch(free)], batch
sharded 8 cores x chunks of 256 columns. PSUM mega tile per layer
[128, 1024] = [z_m0|z_m1|dz_m0|dz_m1]. Matmuls in fp32r (full-rate fp32,
11-bit mantissa, inputs pre-rounded). Biases injected via K=1 ones-row
matmuls so ACT ops span both m-halves in one instruction.

Walrus caps several encodings (fused-LDW matmuls, Drain) at ONE sync
wait and rejects EVENT_SEMAPHORE_RANGE_CLEAR; _legalize_sync post-processes
the scheduled IR into carrier EventSemaphore instructions to satisfy it.
Two chunks are emitted interleaved per step so PE/ACT/DVE overlap across
chunks despite the serial per-chunk dependency chain.
"""

import numpy as np

B, A, F, H, N_STEPS = 32768, 8, 256, 256, 50
N_CORES = 8
B_LOC = B // N_CORES  # 4096
N_COL = 256  # batch columns per chunk
EPS_HALF = 10  # steps per eps load slab

# WPACK column offsets (f32r constants packed into one [128, NW] tensor).
# fp32r matmuls must span all 4 PE column groups, so every stationary is
# padded to M=128 with zero columns.
O_I128 = 0
O_W1C = 128
O_W2 = 640
O_W3V = 1152  # 2 k-tiles x [128,128], W3v in cols 0:8 of each
O_ONES = 1408
O_B2 = 1664
O_I8 = 1920  # [8,128], I8 in cols 0:8
O_WDIV = 2048  # [8,128], col 0 = -1
O_WHALF = 2176  # [8,128], col 0 = 0.5
O_W1A = 2304  # rows 0:8, 256 cols
NW = 2560

_CACHE = {}


def _build(n_steps, n_chunks, legalize=True):
    import concourse.bass as bass
    import concourse.mybir as mybir
    import concourse.tile as tile
    from concourse.alu_op_type import AluOpType

    dt_ = mybir.dt
    AF = mybir.ActivationFunctionType
    f32 = dt_.float32
    f32r = dt_.float32r

    nc = bass.Bass()

    # ---- DRAM params (per-core; weights replicated, data sharded) ----
    WPACK = nc.declare_dram_parameter("WPACK", [128, NW], f32r, isOutput=False)
    B1EFF = nc.declare_dram_parameter("B1EFF", [1, n_steps * 256], f32r, isOutput=False)
    B3V = nc.declare_dram_parameter("B3V", [8, 1], f32, isOutput=False)
    CB = nc.declare_dram_parameter("CB", [1, 1], f32, isOutput=False)
    CT = nc.declare_dram_parameter("CT", [256, n_chunks * N_COL], f32r, isOutput=False)
    ACT8 = nc.declare_dram_parameter("ACT8", [8, n_chunks * N_COL], f32r, isOutput=False)
    EPS = nc.declare_dram_parameter(
        "EPS", [n_steps, 8, n_chunks * N_COL], f32r, isOutput=False
    )
    OUT = nc.declare_dram_parameter("OUT", [1, n_chunks * N_COL], f32, isOutput=True)

    n_half = max(1, n_steps // EPS_HALF) if n_steps >= EPS_HALF else 1
    eps_half_steps = n_steps // n_half
    assert eps_half_steps * n_half == n_steps

    def mm(out, lhsT, rhs, **kw):
        nc.tensor.matmul(out, lhsT, rhs, skip_group_check=True, **kw)

    with tile.TileContext(nc) as tc:
        with (
            tc.tile_pool(name="wpool", bufs=1) as wpool,
            tc.tile_pool(name="cpool", bufs=3) as cpool,
            tc.tile_pool(name="epool", bufs=3) as epool,
            tc.tile_pool(name="apool", bufs=6) as apool,
            tc.tile_pool(name="hpool", bufs=4) as hpool,
            tc.tile_pool(name="spool", bufs=4) as spool,
            tc.tile_pool(name="opool", bufs=1) as opool,
            tc.tile_pool(name="pmega", bufs=2, space="PSUM") as pmega,
            tc.tile_pool(name="pvj", bufs=3, space="PSUM") as pvj,
            tc.tile_pool(name="pdiv", bufs=1, space="PSUM") as pdiv,
        ):
            # ---- load constants (single DMA for all matmul-feeding consts) ----
            wp = wpool.tile([128, NW], f32r, name="wp")
            nc.sync.dma_start(out=wp, in_=WPACK[:, :])
            b1e = wpool.tile([1, n_steps * 256], f32r, name="b1e")
            nc.sync.dma_start(out=b1e, in_=B1EFF[:, :])
            b3v = wpool.tile([8, 1], f32, name="b3v")
            nc.sync.dma_start(out=b3v, in_=B3V[:, :])
            cb = wpool.tile([1, 1], f32, name="cb")
            nc.sync.dma_start(out=cb, in_=CB[:, :])

            i128 = wp[:, O_I128 : O_I128 + 128]
            ones = wp[0:1, O_ONES : O_ONES + N_COL]
            b2r = wp[0:1, O_B2 : O_B2 + 256]
            i8pad = wp[0:8, O_I8 : O_I8 + 128]
            wdiv = wp[0:8, O_WDIV : O_WDIV + 128]
            whalf = wp[0:8, O_WHALF : O_WHALF + 128]
            w1a = wp[0:8, O_W1A : O_W1A + 256]

            def w1c(k, m):
                return wp[:, O_W1C + k * 256 + m * 128 : O_W1C + k * 256 + (m + 1) * 128]

            def w2s(k, m):
                return wp[:, O_W2 + k * 256 + m * 128 : O_W2 + k * 256 + (m + 1) * 128]

            def w3vs(k):
                return wp[:, O_W3V + k * 128 : O_W3V + (k + 1) * 128]

            out_acc = opool.tile([1, n_chunks * N_COL], f32, name="out_acc")

            assert n_chunks % 2 == 0
            for pair in range(n_chunks // 2):
                pcols = slice(pair * 2 * N_COL, (pair + 1) * 2 * N_COL)

                div_ps = pdiv.tile([128, 2 * N_COL], f32, tag="div", name="div_ps")

                # ---- pair setup: actor features, zc for both chunks ----
                ct = cpool.tile([128, 2, 2 * N_COL], f32r, tag="ct", name="ct")
                for k in range(2):
                    nc.sync.dma_start(
                        out=ct[:, k, :], in_=CT[k * 128 : (k + 1) * 128, pcols]
                    )
                # a_es tiles: [8, 2, 256] = [a | e_s]; a from ACT8/ACT-copy,
                # es DMA'd per step. Combined they form one K=8 moving operand
                # that produces z's a-part AND dz in a single matmul.
                a_st = []
                for half in range(2):
                    ccols = slice((pair * 2 + half) * N_COL, (pair * 2 + half + 1) * N_COL)
                    av = apool.tile([8, 2, N_COL], f32r, tag="aes", name="aes_init")
                    nc.sync.dma_start(out=av[:, 0, :], in_=ACT8[:, ccols])
                    nc.sync.dma_start(out=av[:, 1, :], in_=EPS[0, :, ccols])
                    a_st.append(av)

                # zc = W1c^T c per chunk, laid out [A: m0|m1 | B: m0|m1]
                zc_ps = pmega.tile([128, 1024], f32, tag="mega", name="zc_ps")
                for half in range(2):
                    for m in range(2):
                        o = half * 512 + m * 256
                        for k in range(2):
                            mm(
                                zc_ps[:, o : o + 256],
                                w1c(k, m),
                                ct[:, k, half * 256 : (half + 1) * 256],
                                start=(k == 0),
                                stop=(k == 1),
                            )
                zc = hpool.tile([128, 1024], f32r, tag="zc", name="zc")
                nc.scalar.copy(zc, zc_ps)

                # ---- Euler steps, two chunks interleaved ----
                for s in range(n_steps):
                    bcol = s * 256
                    for half in range(2):
                        ch = pair * 2 + half
                        ccols = slice(ch * N_COL, (ch + 1) * N_COL)
                        aes = a_st[half]
                        es = aes[:, 1, :]

                        # ----- layer 1: ZZ[p, m, z|dz, j] -----
                        ZZ = pmega.tile([128, 2, 2, 256], f32, tag="mega", name="zz1")
                        for m in range(2):
                            mm(
                                ZZ[:, m],
                                w1a[:, m * 128 : (m + 1) * 128],
                                aes[0:8],
                                start=True,
                                stop=False,
                            )
                            mm(
                                ZZ[:, m, 0],
                                i128,
                                zc[:, half * 512 + m * 256 : half * 512 + (m + 1) * 256],
                                start=False,
                                stop=False,
                            )
                            mm(
                                ZZ[:, m, 0],
                                b1e[0:1, bcol + m * 128 : bcol + (m + 1) * 128],
                                ones,
                                start=False,
                                stop=True,
                            )
                        # hd: [128, h|dh, k, 256]; h = silu(z), dh = silu'(z)*dz
                        hd1 = hpool.tile([128, 2, 2, 256], f32r, tag="hd", name="hd1")
                        t1 = hpool.tile([128, 2, 256], f32, tag="t", name="t1")
                        nc.scalar.activation(hd1[:, 0], ZZ[:, :, 0, :], AF.Silu)
                        nc.scalar.activation(t1, ZZ[:, :, 0, :], AF.Tanh, scale=0.5)
                        q1 = hpool.tile([128, 2, 256], f32, tag="q", name="q1")
                        nc.vector.tensor_scalar(q1, t1, -0.5, 0.5, AluOpType.mult, AluOpType.add)
                        m1 = hpool.tile([128, 2, 256], f32, tag="m", name="m1")
                        nc.vector.tensor_scalar_add(m1, hd1[:, 0].bitcast(f32), -1.0)
                        r1 = hpool.tile([128, 2, 256], f32, tag="r", name="r1")
                        nc.vector.tensor_tensor(r1, q1, m1, AluOpType.mult)
                        nc.vector.scalar_tensor_tensor(
                            hd1[:, 1], r1, 1.0, ZZ[:, :, 1, :], AluOpType.add, AluOpType.mult
                        )

                        # ----- layer 2 -----
                        Z2 = pmega.tile([128, 2, 2, 256], f32, tag="mega", name="zz2")
                        for m in range(2):
                            for k in range(2):
                                mm(
                                    Z2[:, m],
                                    w2s(k, m),
                                    hd1[:, :, k, :],
                                    start=(k == 0),
                                    stop=False,
                                )
                            mm(
                                Z2[:, m, 0],
                                b2r[0:1, m * 128 : (m + 1) * 128],
                                ones,
                                start=False,
                                stop=True,
                            )
                        hd2 = hpool.tile([128, 2, 2, 256], f32r, tag="hd", name="hd2")
                        t2 = hpool.tile([128, 2, 256], f32, tag="t", name="t2")
                        nc.scalar.activation(hd2[:, 0], Z2[:, :, 0, :], AF.Silu)
                        nc.scalar.activation(t2, Z2[:, :, 0, :], AF.Tanh, scale=0.5)
                        q2 = hpool.tile([128, 2, 256], f32, tag="q", name="q2")
                        nc.vector.tensor_scalar(q2, t2, -0.5, 0.5, AluOpType.mult, AluOpType.add)
                        m2 = hpool.tile([128, 2, 256], f32, tag="m", name="m2")
                        nc.vector.tensor_scalar_add(m2, hd2[:, 0].bitcast(f32), -1.0)
                        r2 = hpool.tile([128, 2, 256], f32, tag="r", name="r2")
                        nc.vector.tensor_tensor(r2, q2, m2, AluOpType.mult)
                        nc.vector.scalar_tensor_tensor(
                            hd2[:, 1], r2, 1.0, Z2[:, :, 1, :], AluOpType.add, AluOpType.mult
                        )

                        # ----- layer 3: [v|jv] in one group, a update, div -----
                        VJ = pvj.tile([128, 2, 256], f32, tag="pvj", name="vj")
                        for k in range(2):
                            mm(
                                VJ[:, :, :],
                                w3vs(k),
                                hd2[:, :, k, :],
                                start=(k == 0),
                                stop=False,
                            )
                        mm(VJ[:, 0], i8pad, aes[0:8, 0, :], start=False, stop=True)
                        a_new = apool.tile([8, 2, N_COL], f32r, tag="aes", name="a_new")
                        nc.scalar.activation(
                            a_new[:, 0, :], VJ[0:8, 0, :], AF.Identity, bias=b3v[0:8, 0:1]
                        )
                        if s + 1 < n_steps:
                            nc.sync.dma_start(out=a_new[:, 1, :], in_=EPS[s + 1, :, ccols])
                        tmp = spool.tile([8, N_COL], f32r, tag="tmp", name="tmp")
                        nc.vector.tensor_tensor(
                            tmp, VJ[0:8, 1, :], es.bitcast(f32), AluOpType.mult
                        )
                        mm(
                            div_ps[:, half * N_COL : (half + 1) * N_COL],
                            wdiv,
                            tmp,
                            start=(s == 0 and half == 0),
                            stop=False,
                        )
                        a_st[half] = a_new

                # ---- pair finalize ----
                for half in range(2):
                    ch = pair * 2 + half
                    cols = slice(ch * N_COL, (ch + 1) * N_COL)
                    sq = spool.tile([8, N_COL], f32r, tag="tmp", name="sq")
                    nc.scalar.square(sq, a_st[half][0:8, 0, :].bitcast(f32))
                    mm(
                        div_ps[:, half * N_COL : (half + 1) * N_COL],
                        whalf,
                        sq,
                        start=False,
                        stop=True,
                    )
                    nc.scalar.activation(
                        out_acc[0:1, cols],
                        div_ps[0:1, half * N_COL : (half + 1) * N_COL],
                        AF.Identity,
                        bias=cb[0:1, 0:1],
                        scale=-1.0,
                    )

            nc.sync.dma_start(out=OUT[:, :], in_=out_acc)

    return _legalize_sync(nc) if legalize else nc


def _legalize_sync(nc):
    """Post-Tile IR pass for this walrus build's sync limits.

    - EVENT_SEMAPHORE_RANGE_CLEAR (InstISA op 176) is rejected outright
      ("ISA wrong length"); expand it into per-sem EventSemaphore
      `sem-wr-imm 0` resets.
    - Several instruction encodings accept only ONE sync wait (fused-LDW
      matmul, Drain, ...); hoist all but the last wait onto single-wait
      EventSemaphore carriers placed immediately before on the same engine
      (waiting earlier is always sound).
    """
    import concourse.mybir as mybir

    for fn in nc.m.functions:
        for blk in fn.blocks:
            new = []
            for inst in blk.instructions:
                si = getattr(inst, "sync_info", None)
                waits = list(si.on_wait) if si and si.on_wait else []
                updates = list(si.on_update) if si and si.on_update else []

                if (
                    type(inst).__name__ == "InstISA"
                    and getattr(inst, "op_name", None) == "EVENT_SEMAPHORE_RANGE_CLEAR"
                ):
                    d = inst.ant_dict
                    for w in waits:
                        new.append(
                            mybir.InstEventSemaphore(
                                name=f"{inst.name}w{len(new)}",
                                engine=inst.engine,
                                ins=[],
                                outs=[],
                                sync_info=mybir.SyncInfo(on_wait=[w], on_update=[]),
                            )
                        )
                    resets = [
                        mybir.SyncUpdate(
                            sync_type="semaphore",
                            id=sem,
                            update_mode="sem-wr-imm",
                            update_value=0,
                            ant_name=f"rc_{sem}",
                        )
                        for sem in range(d["range_first"], d["range_last"] + 1)
                    ] + updates
                    for j, u in enumerate(resets):
                        new.append(
                            mybir.InstEventSemaphore(
                                name=f"{inst.name}u{j}",
                                engine=inst.engine,
                                ins=[],
                                outs=[],
                                sync_info=mybir.SyncInfo(on_wait=[], on_update=[u]),
                            )
                        )
                    continue

                if len(waits) > 1:
                    for j, w in enumerate(waits[:-1]):
                        new.append(
                            mybir.InstEventSemaphore(
                                name=f"{inst.name}w{j}",
                                engine=inst.engine,
                                ins=[],
                                outs=[],
                                sync_info=mybir.SyncInfo(on_wait=[w], on_update=[]),
                            )
                        )
                    inst.sync_info = mybir.SyncInfo(
                        on_wait=[waits[-1]], on_update=updates
                    )
                new.append(inst)
            blk.instructions = new
    return nc


def _r32r(x):
    """Round fp32 -> fp32r (11-bit mantissa, RNE at bit 12). Matches walrus
    fp32_to_fp32r bit-exactly on non-NaN/Inf inputs."""
    x = np.ascontiguousarray(x, np.float32)
    u = x.view(np.uint32).astype(np.uint64)
    u = (u + 0x7FF + ((u >> 12) & 1)) & 0xFFFFF000
    return u.astype(np.uint32).view(np.float32)


def _host_prep(actions, actor_features, W1, b1, W2, b2, W3, b3, eps):
    """Full-input host-side prep -> per-core input maps."""
    n_steps = eps.shape[0]
    dt = 1.0 / n_steps
    t_vals = (1.0 - np.arange(n_steps, dtype=np.float32) * np.float32(dt)).astype(
        np.float32
    )

    W1 = np.asarray(W1, np.float32)
    W1a = W1[0:A, :]  # [8,256]
    W1c = W1[A : A + F, :]  # [256,256]
    w1t = W1[A + F, :]  # [256]
    b1 = np.asarray(b1, np.float32)
    W2 = np.asarray(W2, np.float32)
    b2 = np.asarray(b2, np.float32)
    W3 = np.asarray(W3, np.float32)
    b3 = np.asarray(b3, np.float32)

    wpack = np.zeros((128, NW), np.float32)
    wpack[:, O_I128 : O_I128 + 128] = np.eye(128, dtype=np.float32)
    for k in range(2):
        wpack[:, O_W1C + k * 256 : O_W1C + (k + 1) * 256] = W1c[k * 128 : (k + 1) * 128]
        wpack[:, O_W2 + k * 256 : O_W2 + (k + 1) * 256] = W2[k * 128 : (k + 1) * 128]
        wpack[:, O_W3V + k * 128 : O_W3V + k * 128 + 8] = (
            -np.float32(dt) * W3[k * 128 : (k + 1) * 128]
        )
    wpack[0, O_ONES : O_ONES + N_COL] = 1.0
    wpack[64, O_ONES : O_ONES + N_COL] = 1.0
    wpack[0, O_B2 : O_B2 + 256] = b2
    wpack[0:8, O_I8 : O_I8 + 8] = np.eye(8, dtype=np.float32)
    wpack[0:8, O_WDIV] = -1.0
    wpack[0:8, O_WHALF] = 0.5
    wpack[0:8, O_W1A : O_W1A + 256] = W1a
    wpack = _r32r(wpack)

    b1eff = _r32r(b1[None, :] + t_vals[:, None] * w1t[None, :]).reshape(1, -1)  # [1,S*256]

    shared = {
        "WPACK": wpack,
        "B1EFF": b1eff,
        "B3V": np.ascontiguousarray((-np.float32(dt) * b3).reshape(8, 1)),
        "CB": np.full((1, 1), -0.5 * A * np.log(2.0 * np.pi), np.float32),
    }

    bsz = actions.shape[0]
    b_loc = bsz // N_CORES
    act8 = _r32r(np.asarray(actions, np.float32)).T  # [8,B]
    cT = _r32r(np.asarray(actor_features, np.float32).T)  # [256,B]
    epsT = _r32r(np.asarray(eps, np.float32).transpose(0, 2, 1))  # [S,8,B]

    per_core = []
    for c in range(N_CORES):
        sl = slice(c * b_loc, (c + 1) * b_loc)
        m = dict(shared)
        m["ACT8"] = np.ascontiguousarray(act8[:, sl])
        m["CT"] = np.ascontiguousarray(cT[:, sl])
        m["EPS"] = np.ascontiguousarray(epsT[:, :, sl])
        per_core.append(m)
    return per_core


def _run(inputs, trace=False):
    from concourse.bass_utils import run_bass_kernel_spmd

    eps = np.asarray(inputs["eps"], np.float32)
    n_steps = eps.shape[0]
    bsz = np.asarray(inputs["actions"]).shape[0]
    n_chunks = bsz // N_CORES // N_COL

    key = (n_steps, n_chunks)
    if key not in _CACHE:
        _CACHE[key] = _build(n_steps, n_chunks)
    nc = _CACHE[key]

    in_maps = _host_prep(
        inputs["actions"],
        inputs["actor_features"],
        inputs["W1"],
        inputs["b1"],
        inputs["W2"],
        inputs["b2"],
        inputs["W3"],
        inputs["b3"],
        eps,
    )
    res = run_bass_kernel_spmd(nc, in_maps, core_ids=list(range(N_CORES)), trace=trace)
    outs = [res.results[c]["OUT"].reshape(-1) for c in range(N_CORES)]
    full = np.concatenate(outs).astype(np.float32).reshape(bsz, 1)
    return full, res


def kernel(**inputs):
    out, _ = _run(inputs, trace=False)
    return out



# revision 13
# speedup vs baseline: 1.5367x; 1.5367x over previous
"""Trainium2 Bass kernel: FlowMatching action-distribution log-prob head.

Math (per Euler step s, t_s = 1 - s*dt, dt = 1/n_steps):
    z1 = a@W1a + c@W1c + t_s*w1t + b1          (W1 split: rows 0:8 / 8:264 / 264)
    h1 = silu(z1);  dz1 = e@W1a;  dh1 = silu'(z1) * dz1
    z2 = h1@W2 + b2; h2 = silu(z2); dz2 = dh1@W2; dh2 = silu'(z2)*dz2
    v  = h2@W3 + b3; jv = dh2@W3
    a -= v*dt;  div_int += dt * sum(jv*e)
Output: logp = -0.5*||a0||^2 - 0.5*A*ln(2pi) - div_int     [B,1]

silu and derivative_silu live in different ACT table sets on TRN2 (~2.7us
switch per use), so silu' is computed exactly from the silu set itself:
    T = tanh(z/2);  q = (1-T)/2;  silu'(z) = 1 + q*(silu(z)-1)
The tangent rides through W3V = -dt*W3:
    jv~ = dh2 @ W3V = -dt*jv;  tmp = jv~*e;  div_ps += (-1)*sum_p tmp = +dt*div_s

Engine assignment per step-chunk (256 batch cols), chosen to balance the
four compute engines (GPSIMD has no PSUM port, so PSUM-reading ops stay on
DVE; ACT carries the two table lookups):
    PE  : all matmuls. z/dz/vj live in separate single-bank PSUM tiles so
          downstream ACT/DVE ops never span two banks (bank-spanning ops
          get split in walrus, doubling their fixed overheads).
    ACT : h = silu(z) [f32r], T = tanh(z/2) [bf16]
    DVE : q = -T/2+1/2 [bf16, 4x mode], dh = (r+1)*dz [PSUM],
          a_new = v_psum + b3v, tmp = jv~*e
    POOL: r = (h-1)*q   (SBUF-only fused scalar_tensor_tensor)

Two chunks are processed interleaved, with ops emitted in dependency waves
(alternating chunks wave by wave) so each engine's strict-FIFO queue never
head-of-line blocks on the other chunk's not-yet-ready work. The v-half of
layer 3 is emitted separately from (and ahead of) the jv-half so the serial
a-recurrence (a -> z1 -> h1 -> z2 -> h2 -> v -> a_new) never waits on the
tangent stream. eps is loaded in 10-step slabs (double-buffered) instead of
per-step DMAs.

Walrus caps several encodings (fused-LDW matmuls, Drain) at ONE sync
wait and rejects EVENT_SEMAPHORE_RANGE_CLEAR; _legalize_sync post-processes
the scheduled IR into carrier EventSemaphore instructions to satisfy it.
"""

import numpy as np

B, A, F, H, N_STEPS = 32768, 8, 256, 256, 50
N_CORES = 8
B_LOC = B // N_CORES  # 4096
N_COL = 256  # batch columns per chunk
EPS_SLAB = 10  # steps per eps DMA slab

# WPACK column offsets (f32r constants packed into one [128, NW] tensor).
# fp32r matmuls must span all 4 PE column groups, so every stationary is
# padded to M=128 with zero columns.
O_I128 = 0
O_W1C = 128
O_W2 = 640
O_W3V = 1152  # 2 k-tiles x [128,128], W3v in cols 0:8 of each
O_ONES = 1408
O_B2 = 1664
O_I8 = 1920  # [8,128], I8 in cols 0:8
O_WDIV = 2048  # [8,128], col 0 = -1
O_WHALF = 2176  # [8,128], col 0 = 0.5
O_W1A = 2304  # rows 0:8, 256 cols
NW = 2560

_CACHE = {}


def _build(n_steps, n_chunks, legalize=True):
    import concourse.bass as bass
    import concourse.mybir as mybir
    import concourse.tile as tile
    from concourse.alu_op_type import AluOpType

    dt_ = mybir.dt
    AF = mybir.ActivationFunctionType
    f32 = dt_.float32
    f32r = dt_.float32r
    bf16 = dt_.bfloat16

    nc = bass.Bass()

    # ---- DRAM params (per-core; weights replicated, data sharded) ----
    WPACK = nc.declare_dram_parameter("WPACK", [128, NW], f32r, isOutput=False)
    B1EFF = nc.declare_dram_parameter("B1EFF", [1, n_steps * 256], f32r, isOutput=False)
    B3V = nc.declare_dram_parameter("B3V", [8, 1], f32, isOutput=False)
    CB = nc.declare_dram_parameter("CB", [1, 1], f32, isOutput=False)
    CT = nc.declare_dram_parameter("CT", [256, n_chunks * N_COL], f32r, isOutput=False)
    ACT8 = nc.declare_dram_parameter("ACT8", [8, n_chunks * N_COL], f32r, isOutput=False)
    # eps laid out action-major so a 10-step slab for one chunk is a clean
    # [8, slab*256] strided read
    EPS = nc.declare_dram_parameter(
        "EPS", [8, n_steps, n_chunks * N_COL], f32r, isOutput=False
    )
    OUT = nc.declare_dram_parameter("OUT", [1, n_chunks * N_COL], f32, isOutput=True)

    n_slabs = (n_steps + EPS_SLAB - 1) // EPS_SLAB
    assert n_slabs * EPS_SLAB == n_steps

    def mm(out, lhsT, rhs, **kw):
        nc.tensor.matmul(out, lhsT, rhs, skip_group_check=True, **kw)

    with tile.TileContext(nc) as tc:
        with (
            tc.tile_pool(name="wpool", bufs=1) as wpool,
            tc.tile_pool(name="cpool", bufs=2) as cpool,
            tc.tile_pool(name="epool", bufs=2) as epool,
            tc.tile_pool(name="apool", bufs=2) as apool,
            tc.tile_pool(name="hpool", bufs=2) as hpool,
            tc.tile_pool(name="spool", bufs=2) as spool,
            tc.tile_pool(name="opool", bufs=1) as opool,
            tc.tile_pool(name="pring", bufs=4, space="PSUM") as pring,
            tc.tile_pool(name="pdiv", bufs=1, space="PSUM") as pdiv,
        ):
            # ---- load constants (single DMA for all matmul-feeding consts) ----
            wp = wpool.tile([128, NW], f32r, name="wp")
            nc.sync.dma_start(out=wp, in_=WPACK[:, :])
            b1e = wpool.tile([1, n_steps * 256], f32r, name="b1e")
            nc.sync.dma_start(out=b1e, in_=B1EFF[:, :])
            b3v = wpool.tile([8, 1], f32, name="b3v")
            nc.sync.dma_start(out=b3v, in_=B3V[:, :])
            cb = wpool.tile([1, 1], f32, name="cb")
            nc.sync.dma_start(out=cb, in_=CB[:, :])

            i128 = wp[:, O_I128 : O_I128 + 128]
            ones = wp[0:1, O_ONES : O_ONES + N_COL]
            b2r = wp[0:1, O_B2 : O_B2 + 256]
            i8pad = wp[0:8, O_I8 : O_I8 + 128]
            wdiv = wp[0:8, O_WDIV : O_WDIV + 128]
            whalf = wp[0:8, O_WHALF : O_WHALF + 128]
            w1a = wp[0:8, O_W1A : O_W1A + 256]

            def w1c(k, m):
                return wp[:, O_W1C + k * 256 + m * 128 : O_W1C + k * 256 + (m + 1) * 128]

            def w2s(k, m):
                return wp[:, O_W2 + k * 256 + m * 128 : O_W2 + k * 256 + (m + 1) * 128]

            def w3vs(k):
                return wp[:, O_W3V + k * 128 : O_W3V + (k + 1) * 128]

            out_acc = opool.tile([1, n_chunks * N_COL], f32, name="out_acc")

            assert n_chunks % 2 == 0
            for pair in range(n_chunks // 2):
                pcols = slice(pair * 2 * N_COL, (pair + 1) * 2 * N_COL)

                div_ps = pdiv.tile([128, 2 * N_COL], f32, tag="div", name="div_ps")

                # ---- pair setup: actor features, zc, a0, first eps slab ----
                ct = cpool.tile([128, 2, 2 * N_COL], f32r, tag="ct", name="ct")
                for k in range(2):
                    nc.sync.dma_start(
                        out=ct[:, k, :], in_=CT[k * 128 : (k + 1) * 128, pcols]
                    )
                a_st, e_st = [], [[None] * 2, [None] * 2]
                for half in range(2):
                    ccols = slice((pair * 2 + half) * N_COL, (pair * 2 + half + 1) * N_COL)
                    av = apool.tile([8, N_COL], f32r, tag=f"a{half}", bufs=2, name="a0")
                    nc.sync.dma_start(out=av, in_=ACT8[:, ccols])
                    a_st.append(av)
                    ev = epool.tile(
                        [8, EPS_SLAB, N_COL], f32r, tag=f"e{half}", bufs=2, name="eps"
                    )
                    nc.sync.dma_start(out=ev, in_=EPS[:, 0:EPS_SLAB, ccols])
                    e_st[half][0] = ev

                # zc = W1c^T c per half, [m0|m1] in one bank; then to SBUF
                zc = cpool.tile([128, 2, 512], f32r, tag="zc", name="zc")
                for half in range(2):
                    zc_ps = pring.tile([128, 512], f32, tag="zzv", bufs=4, name="zc_ps")
                    for m in range(2):
                        for k in range(2):
                            mm(
                                zc_ps[:, m * 256 : (m + 1) * 256],
                                w1c(k, m),
                                ct[:, k, half * 256 : (half + 1) * 256],
                                start=(m == 0 and k == 0),
                                stop=(m == 1 and k == 1),
                            )
                    nc.scalar.copy(zc[:, half, :], zc_ps)

                # ---- Euler steps, two chunks interleaved, wave emission ----
                for s in range(n_steps):
                    bcol = s * 256
                    slab, off = divmod(s, EPS_SLAB)

                    # prefetch next eps slab at slab boundary
                    if off == 0 and (s + EPS_SLAB) < n_steps:
                        for half in range(2):
                            ccols = slice(
                                (pair * 2 + half) * N_COL,
                                (pair * 2 + half + 1) * N_COL,
                            )
                            ev = epool.tile(
                                [8, EPS_SLAB, N_COL],
                                f32r,
                                tag=f"e{half}",
                                bufs=2,
                                name="eps",
                            )
                            nc.sync.dma_start(
                                out=ev,
                                in_=EPS[:, s + EPS_SLAB : s + 2 * EPS_SLAB, ccols],
                            )
                            e_st[half][(slab + 1) % 2] = ev

                    es = [e_st[half][slab % 2][:, off, :] for half in range(2)]

                    # ----- wave: a-independent matmuls (zc/b1/b2 first so the
                    # critical a-matmul is the only thing gating silu1).
                    # These share PSUM accumulation ranges with the
                    # high-priority critical matmuls below; the Tile scheduler
                    # treats accumulating writes as commutative, so they MUST
                    # carry the same priority or the range-resetting
                    # (start=True) op can be scheduled after the accumulate
                    # ops. high_priority on everything in a group keeps the
                    # emission order authoritative.
                    Z1s, Z2s = [], []
                    with tc.high_priority():
                        for half in range(2):
                            Z1 = pring.tile([128, 512], f32, tag="zzv", bufs=4,
                                            name="z1")
                            for m in range(2):
                                mm(Z1[:, m * 256 : (m + 1) * 256], i128,
                                   zc[:, half, m * 256 : (m + 1) * 256],
                                   start=(m == 0), stop=False)
                            for m in range(2):
                                mm(Z1[:, m * 256 : (m + 1) * 256],
                                   b1e[0:1, bcol + m * 128 : bcol + (m + 1) * 128],
                                   ones, start=False, stop=False)
                            Z1s.append(Z1)
                        for half in range(2):
                            Z2 = pring.tile([128, 512], f32, tag="zzv", bufs=4,
                                            name="z2")
                            for m in range(2):
                                mm(Z2[:, m * 256 : (m + 1) * 256],
                                   b2r[0:1, m * 128 : (m + 1) * 128],
                                   ones, start=(m == 0), stop=False)
                            Z2s.append(Z2)

                    # ----- critical path: a-mms -> silu1 -> L2z -> silu2 -> v -> a_new
                    # high_priority so the scheduler always prefers these over
                    # same-readiness tangent ops on every engine
                    with tc.high_priority():
                        for half in range(2):
                            for m in range(2):
                                wm = w1a[:, m * 128 : (m + 1) * 128]
                                mm(Z1s[half][:, m * 256 : (m + 1) * 256], wm,
                                   a_st[half], start=False, stop=(m == 1))
                        h1s = []
                        for half in range(2):
                            h1 = hpool.tile([128, 512], f32r, tag="h", bufs=4,
                                            name="h1")
                            nc.scalar.activation(h1, Z1s[half], AF.Silu)
                            h1s.append(h1)
                        for half in range(2):
                            for m in range(2):
                                for k in range(2):
                                    mm(Z2s[half][:, m * 256 : (m + 1) * 256],
                                       w2s(k, m),
                                       h1s[half][:, k * 256 : (k + 1) * 256],
                                       start=False, stop=(k == 1))
                        h2s = []
                        for half in range(2):
                            h2 = hpool.tile([128, 512], f32r, tag="h", bufs=4,
                                            name="h2")
                            nc.scalar.activation(h2, Z2s[half], AF.Silu)
                            h2s.append(h2)
                        VJs = []
                        for half in range(2):
                            VJ = pring.tile([128, 512], f32, tag="zzv", bufs=4,
                                            name="vj")
                            for k in range(2):
                                mm(VJ[:, 0:256], w3vs(k),
                                   h2s[half][:, k * 256 : (k + 1) * 256],
                                   start=(k == 0), stop=False)
                            mm(VJ[:, 0:256], i8pad, a_st[half],
                               start=False, stop=True)
                            VJs.append(VJ)
                        for half in range(2):
                            a_new = apool.tile([8, N_COL], f32r, tag=f"a{half}",
                                               bufs=2, name="a_new")
                            nc.scalar.activation(a_new, VJs[half][0:8, 0:256],
                                                 AF.Identity, bias=b3v[0:8, 0:1])
                            a_st[half] = a_new

                    # ----- tangent stream (off the critical path, deferred) -----
                    t1s, t2s = [], []
                    for half in range(2):
                        t1 = hpool.tile([128, 512], bf16, tag="t", bufs=4, name="t1")
                        nc.scalar.activation(t1, Z1s[half], AF.Tanh, scale=0.5)
                        t1s.append(t1)
                    for half in range(2):
                        t2 = hpool.tile([128, 512], bf16, tag="t", bufs=4, name="t2")
                        nc.scalar.activation(t2, Z2s[half], AF.Tanh, scale=0.5)
                        t2s.append(t2)
                    DZ1s = []
                    for half in range(2):
                        DZ1 = pring.tile([128, 512], f32, tag="dz", bufs=3, name="dz1")
                        for m in range(2):
                            wm = w1a[:, m * 128 : (m + 1) * 128]
                            mm(DZ1[:, m * 256 : (m + 1) * 256], wm, es[half],
                               start=(m == 0), stop=(m == 1))
                        DZ1s.append(DZ1)
                    q1s, r1s, dh1s = [], [], []
                    for half in range(2):
                        q1 = hpool.tile([128, 512], bf16, tag="q", bufs=4, name="q1")
                        nc.vector.tensor_scalar(q1, t1s[half], -0.5, 0.5,
                                                AluOpType.mult, AluOpType.add)
                        q1s.append(q1)
                    m1s = []
                    for half in range(2):
                        m1 = hpool.tile([128, 512], bf16, tag="m", bufs=4, name="m1")
                        nc.gpsimd.tensor_scalar_add(m1, h1s[half], -1.0)
                        m1s.append(m1)
                    for half in range(2):
                        r1 = hpool.tile([128, 512], bf16, tag="r", bufs=4, name="r1")
                        nc.vector.tensor_tensor(r1, m1s[half], q1s[half],
                                                AluOpType.mult)
                        r1s.append(r1)
                    for half in range(2):
                        dh1 = hpool.tile([128, 512], f32r, tag="dh", bufs=4, name="dh1")
                        nc.vector.scalar_tensor_tensor(
                            dh1, r1s[half], 1.0, DZ1s[half],
                            AluOpType.add, AluOpType.mult)
                        dh1s.append(dh1)
                    DZ2s = []
                    for half in range(2):
                        DZ2 = pring.tile([128, 512], f32, tag="dz", bufs=3, name="dz2")
                        for m in range(2):
                            for k in range(2):
                                mm(DZ2[:, m * 256 : (m + 1) * 256], w2s(k, m),
                                   dh1s[half][:, k * 256 : (k + 1) * 256],
                                   start=(m == 0 and k == 0),
                                   stop=(m == 1 and k == 1))
                        DZ2s.append(DZ2)
                    q2s, r2s, dh2s = [], [], []
                    for half in range(2):
                        q2 = hpool.tile([128, 512], bf16, tag="q", bufs=4, name="q2")
                        nc.vector.tensor_scalar(q2, t2s[half], -0.5, 0.5,
                                                AluOpType.mult, AluOpType.add)
                        q2s.append(q2)
                    m2s = []
                    for half in range(2):
                        m2 = hpool.tile([128, 512], bf16, tag="m", bufs=4, name="m2")
                        nc.gpsimd.tensor_scalar_add(m2, h2s[half], -1.0)
                        m2s.append(m2)
                    for half in range(2):
                        r2 = hpool.tile([128, 512], bf16, tag="r", bufs=4, name="r2")
                        nc.vector.tensor_tensor(r2, m2s[half], q2s[half],
                                                AluOpType.mult)
                        r2s.append(r2)
                    for half in range(2):
                        dh2 = hpool.tile([128, 512], f32r, tag="dh", bufs=4, name="dh2")
                        nc.vector.scalar_tensor_tensor(
                            dh2, r2s[half], 1.0, DZ2s[half],
                            AluOpType.add, AluOpType.mult)
                        dh2s.append(dh2)
                    for half in range(2):
                        for k in range(2):
                            mm(VJs[half][:, 256:512], w3vs(k),
                               dh2s[half][:, k * 256 : (k + 1) * 256],
                               start=False, stop=(k == 1))
                    for half in range(2):
                        tmp = spool.tile([8, N_COL], f32r, tag="tmp", bufs=4,
                                         name="tmp")
                        nc.vector.tensor_tensor(tmp, VJs[half][0:8, 256:512],
                                                es[half], AluOpType.mult)
                        mm(div_ps[:, half * N_COL : (half + 1) * N_COL], wdiv, tmp,
                           start=(s == 0 and half == 0), stop=False)

                # ---- pair finalize ----
                for half in range(2):
                    ch = pair * 2 + half
                    cols = slice(ch * N_COL, (ch + 1) * N_COL)
                    sq = spool.tile([8, N_COL], f32r, tag="tmp", bufs=4, name="sq")
                    nc.scalar.square(sq, a_st[half].bitcast(f32))
                    mm(div_ps[:, half * N_COL : (half + 1) * N_COL], whalf, sq,
                       start=False, stop=True)
                    nc.scalar.activation(
                        out_acc[0:1, cols],
                        div_ps[0:1, half * N_COL : (half + 1) * N_COL],
                        AF.Identity,
                        bias=cb[0:1, 0:1],
                        scale=-1.0,
                    )

            nc.sync.dma_start(out=OUT[:, :], in_=out_acc)

    return _legalize_sync(nc) if legalize else nc


def _legalize_sync(nc):
    """Post-Tile IR pass for this walrus build's sync limits.

    - EVENT_SEMAPHORE_RANGE_CLEAR (InstISA op 176) is rejected outright
      ("ISA wrong length"); expand it into per-sem EventSemaphore
      `sem-wr-imm 0` resets.
    - Several instruction encodings accept only ONE sync wait (fused-LDW
      matmul, Drain, ...); hoist all but the last wait onto single-wait
      EventSemaphore carriers placed immediately before on the same engine
      (waiting earlier is always sound).
    """
    import concourse.mybir as mybir

    for fn in nc.m.functions:
        for blk in fn.blocks:
            new = []
            for inst in blk.instructions:
                si = getattr(inst, "sync_info", None)
                waits = list(si.on_wait) if si and si.on_wait else []
                updates = list(si.on_update) if si and si.on_update else []

                if (
                    type(inst).__name__ == "InstISA"
                    and getattr(inst, "op_name", None) == "EVENT_SEMAPHORE_RANGE_CLEAR"
                ):
                    d = inst.ant_dict
                    for w in waits:
                        new.append(
                            mybir.InstEventSemaphore(
                                name=f"{inst.name}w{len(new)}",
                                engine=inst.engine,
                                ins=[],
                                outs=[],
                                sync_info=mybir.SyncInfo(on_wait=[w], on_update=[]),
                            )
                        )
                    resets = [
                        mybir.SyncUpdate(
                            sync_type="semaphore",
                            id=sem,
                            update_mode="sem-wr-imm",
                            update_value=0,
                            ant_name=f"rc_{sem}",
                        )
                        for sem in range(d["range_first"], d["range_last"] + 1)
                    ] + updates
                    for j, u in enumerate(resets):
                        new.append(
                            mybir.InstEventSemaphore(
                                name=f"{inst.name}u{j}",
                                engine=inst.engine,
                                ins=[],
                                outs=[],
                                sync_info=mybir.SyncInfo(on_wait=[], on_update=[u]),
                            )
                        )
                    continue

                if len(waits) > 1:
                    for j, w in enumerate(waits[:-1]):
                        new.append(
                            mybir.InstEventSemaphore(
                                name=f"{inst.name}w{j}",
                                engine=inst.engine,
                                ins=[],
                                outs=[],
                                sync_info=mybir.SyncInfo(on_wait=[w], on_update=[]),
                            )
                        )
                    inst.sync_info = mybir.SyncInfo(
                        on_wait=[waits[-1]], on_update=updates
                    )
                new.append(inst)
            blk.instructions = new
    return nc


def _r32r(x):
    """Round fp32 -> fp32r (11-bit mantissa, RNE at bit 12). Matches walrus
    fp32_to_fp32r bit-exactly on non-NaN/Inf inputs."""
    x = np.ascontiguousarray(x, np.float32)
    u = x.view(np.uint32).astype(np.uint64)
    u = (u + 0x7FF + ((u >> 12) & 1)) & 0xFFFFF000
    return u.astype(np.uint32).view(np.float32)


def _host_prep(actions, actor_features, W1, b1, W2, b2, W3, b3, eps):
    """Full-input host-side prep -> per-core input maps."""
    n_steps = eps.shape[0]
    dt = 1.0 / n_steps
    t_vals = (1.0 - np.arange(n_steps, dtype=np.float32) * np.float32(dt)).astype(
        np.float32
    )

    W1 = np.asarray(W1, np.float32)
    W1a = W1[0:A, :]  # [8,256]
    W1c = W1[A : A + F, :]  # [256,256]
    w1t = W1[A + F, :]  # [256]
    b1 = np.asarray(b1, np.float32)
    W2 = np.asarray(W2, np.float32)
    b2 = np.asarray(b2, np.float32)
    W3 = np.asarray(W3, np.float32)
    b3 = np.asarray(b3, np.float32)

    wpack = np.zeros((128, NW), np.float32)
    wpack[:, O_I128 : O_I128 + 128] = np.eye(128, dtype=np.float32)
    for k in range(2):
        wpack[:, O_W1C + k * 256 : O_W1C + (k + 1) * 256] = W1c[k * 128 : (k + 1) * 128]
        wpack[:, O_W2 + k * 256 : O_W2 + (k + 1) * 256] = W2[k * 128 : (k + 1) * 128]
        wpack[:, O_W3V + k * 128 : O_W3V + k * 128 + 8] = (
            -np.float32(dt) * W3[k * 128 : (k + 1) * 128]
        )
    wpack[0, O_ONES : O_ONES + N_COL] = 1.0
    wpack[64, O_ONES : O_ONES + N_COL] = 1.0
    wpack[0, O_B2 : O_B2 + 256] = b2
    wpack[0:8, O_I8 : O_I8 + 8] = np.eye(8, dtype=np.float32)
    wpack[0:8, O_WDIV] = -1.0
    wpack[0:8, O_WHALF] = 0.5
    wpack[0:8, O_W1A : O_W1A + 256] = W1a
    wpack = _r32r(wpack)

    b1eff = _r32r(b1[None, :] + t_vals[:, None] * w1t[None, :]).reshape(1, -1)  # [1,S*256]

    shared = {
        "WPACK": wpack,
        "B1EFF": b1eff,
        "B3V": np.ascontiguousarray((-np.float32(dt) * b3).reshape(8, 1)),
        "CB": np.full((1, 1), -0.5 * A * np.log(2.0 * np.pi), np.float32),
    }

    bsz = actions.shape[0]
    b_loc = bsz // N_CORES
    act8 = _r32r(np.asarray(actions, np.float32)).T  # [8,B]
    cT = _r32r(np.asarray(actor_features, np.float32).T)  # [256,B]
    epsT = _r32r(np.asarray(eps, np.float32).transpose(2, 0, 1))  # [8,S,B]

    per_core = []
    for c in range(N_CORES):
        sl = slice(c * b_loc, (c + 1) * b_loc)
        m = dict(shared)
        m["ACT8"] = np.ascontiguousarray(act8[:, sl])
        m["CT"] = np.ascontiguousarray(cT[:, sl])
        m["EPS"] = np.ascontiguousarray(epsT[:, :, sl])
        per_core.append(m)
    return per_core


def _run(inputs, trace=False):
    from concourse.bass_utils import run_bass_kernel_spmd

    eps = np.asarray(inputs["eps"], np.float32)
    n_steps = eps.shape[0]
    bsz = np.asarray(inputs["actions"]).shape[0]
    n_chunks = bsz // N_CORES // N_COL

    key = (n_steps, n_chunks)
    if key not in _CACHE:
        _CACHE[key] = _build(n_steps, n_chunks)
    nc = _CACHE[key]

    in_maps = _host_prep(
        inputs["actions"],
        inputs["actor_features"],
        inputs["W1"],
        inputs["b1"],
        inputs["W2"],
        inputs["b2"],
        inputs["W3"],
        inputs["b3"],
        eps,
    )
    res = run_bass_kernel_spmd(nc, in_maps, core_ids=list(range(N_CORES)), trace=trace)
    outs = [res.results[c]["OUT"].reshape(-1) for c in range(N_CORES)]
    full = np.concatenate(outs).astype(np.float32).reshape(bsz, 1)
    return full, res


def kernel(**inputs):
    out, _ = _run(inputs, trace=False)
    return out
